# revision 11
# baseline (speedup 1.0000x reference)
"""Causal single-head attention (B=4, S=4096, D=1024, H=64) on 8 TRN2 NeuronCores.

Strategy (v2: sequence-parallel with k/v AllGather)
---------------------------------------------------
Data-parallel over batch (2 cores per batch element); within a pair the q rows
are split by 128-row block parity (core even: natural blocks 0,2,..,30; odd:
1,3,..,31), which load-balances the causal triangle.  Each core loads only its
own half of x (bf16, 4 MB), projects q/k/v for its rows, and the pair
exchanges k and v with pipelined AllGathers (8 pieces of 2 blocks).  Each core
then computes COMPLETE attention rows for its own q blocks — no output
combine is needed and the softmax denominator is a per-partition scalar.

Layouts chosen for the TRN2 cost model:
- All matmuls bf16 (1 col/cycle at any width).  kT is [h, kpos] so scores^T
  tiles [128 k, 128 q] come out k-on-partitions, which is exactly the lhsT
  layout the out matmul wants: out_acc[128 q, 65] += exp_tile.T-free via
  lhsT=exp, rhs=v_nat[128 k, 65] — only 65 streamed cols per k block.
- v is projected directly in natural [token, h] layout (x chunk as lhsT), so
  no transposes; col 64 of the exchanged v carries the ones column that
  accumulates the softmax denominator.
- The causal mask is applied ON the tensor engine: an extra accumulating
  matmul lhsT=I, rhs=mask_tile adds -8e9 to the banded/padded scores region.
  Per-core mask data (host-computed) absorbs the even/odd parity asymmetry so
  the SPMD instruction stream is identical on all 8 cores; k/v slots are
  rank-indexed (AllGather order), making every lhsT offset static.
- exp runs on the scalar engine out of PSUM in [128, 8*128] tiles (34 ops
  total) with the 1/8 scale folded in; masked entries underflow to exactly 0,
  matching the reference's -1e9 semantics.  exp outputs bf16.
- Normalization: out_acc col 64 is the denominator (per-partition scalar) →
  DVE reciprocal + tensor_scalar_mul, then direct DMA of final rows.

The host only does layout work plus the fp32→bf16 cast of inputs (~5e-3
worst-case relative error, well inside the 2e-2 gate; flip MM_DT/X_DT to
float32r to recover ~3e-4 at ~1.5x the time).
"""

import numpy as np
from contextlib import ExitStack

import concourse.bass as bass
import concourse.mybir as mybir
import concourse.tile as tile
from concourse import bacc
from concourse.bass_utils import run_bass_kernel_spmd

F32 = mybir.dt.float32
BF16 = mybir.dt.bfloat16

B, S, D, H = 4, 4096, 1024, 64
NCORES = 8
NCH = D // 128        # 8 contraction chunks
NMYB = 16             # my q blocks per core
PIECES = [(0, 2), (2, 2), (4, 4), (8, 4), (12, 4)]  # (first block, nblocks)
NPIECE = len(PIECES)
MASK_VAL = -8.0e9     # raw-score mask; exp(0.125*(s+MASK_VAL)) == 0
SCALE = 0.125         # 1/sqrt(H)
GPT = 8               # score/exp groups per PSUM tile ([128, GPT*128])

def _nk(nb): return 64 * 128 * nb
def _nv(nb): return 128 * 65 * nb
def _nx(nb): return _nk(nb) + _nv(nb)


def build_program(with_cc: bool = True):
    nc = bacc.Bacc(num_devices=NCORES)

    xT = nc.declare_dram_parameter("xT", [D, S // 2], BF16, isOutput=False)
    wall = nc.declare_dram_parameter("wall", [D, 256], BF16, isOutput=False)
    bqk = nc.declare_dram_parameter("bqk", [2 * H, 1], F32, isOutput=False)
    bv = nc.declare_dram_parameter("bv", [1, H], F32, isOutput=False)
    masks = nc.declare_dram_parameter("masks", [2, 128, 128], BF16, isOutput=False)
    ident = nc.declare_dram_parameter("ident", [128, 128], BF16, isOutput=False)
    out = nc.declare_dram_parameter("out", [S // 2, H], F32, isOutput=True)

    xT3 = xT.rearrange("(c p) s -> p c s", p=128)        # [128, 8, 2048]
    wall3 = wall.rearrange("(c p) h -> p c h", p=128)    # [128, 8, 256]
    masks3 = masks.rearrange("m p j -> p m j")           # [128, 2, 128]
    out3 = out.rearrange("(n p) h -> p n h", p=128)      # [128, 16, 64]

    with ExitStack() as ctx:
        tc = ctx.enter_context(tile.TileContext(nc))

        singles = ctx.enter_context(tc.tile_pool(name="singles", bufs=1))
        dram = ctx.enter_context(tc.tile_pool(name="dram", bufs=1, space="DRAM"))

        # ---- parameters / constants in SBUF (DVE + Pool queues) ----
        xt_all = singles.tile([128, NCH, S // 2], BF16)
        wall_sb = singles.tile([128, NCH, 256], BF16)
        bqk_sb = singles.tile([2 * H, 1], F32)
        bv_bc = singles.tile([128, H], F32)
        masks_sb = singles.tile([128, 2, 128], BF16)
        ident_sb = singles.tile([128, 128], BF16)
        # small constants on the gpsimd queue (ident first: warmup needs it)
        nc.gpsimd.dma_start(out=ident_sb, in_=ident[:, :])
        nc.gpsimd.dma_start(out=bqk_sb, in_=bqk[:, :])
        nc.gpsimd.dma_start(out=masks_sb, in_=masks3)
        bv_b = bass.AP(tensor=bv[:, :].tensor, offset=bv[:, :].offset,
                       ap=[[0, 128], [1, H]])
        nc.gpsimd.dma_start(out=bv_bc, in_=bv_b)
        # weights + x stream on the scalar queue: none of these wait on
        # anything, so they never clog it, and the sync queue stays free for
        # the latency-critical stage/readback hops
        nc.scalar.dma_start(out=wall_sb, in_=wall3)

        def x_dma(p):
            b0, nb = PIECES[p]
            nc.scalar.dma_start(
                out=xt_all[:, :, b0 * 128 : (b0 + nb) * 128],
                in_=xT3[:, :, b0 * 128 : (b0 + nb) * 128],
            )

        # ---- persistent SBUF state ----
        # rank-indexed k/v for all 32 blocks: slot (r, j) = peer-pair rank r,
        # local block j (natural block 2j + r)
        qT_sb = singles.tile([H, NMYB, 128], BF16)
        kT_sb = singles.tile([H, 2, NMYB, 128], BF16)
        v_sb = singles.tile([128, 2, NMYB, 65], BF16)
        out_stage = singles.tile([128, NMYB, H], F32)
        rcp_sb = singles.tile([128, NMYB], F32)

        # ---- DRAM staging for the k/v AllGather pieces ----
        kv_in = [dram.tile([1, _nx(nb)], BF16, tag=f"kvi{p}", name=f"kv_in{p}")
                 for p, (_, nb) in enumerate(PIECES)]
        kv_red = [dram.tile([2, _nx(nb)], BF16, tag=f"kvr{p}", name=f"kv_red{p}")
                  for p, (_, nb) in enumerate(PIECES)]

        # ---- pools ----
        pj = ctx.enter_context(tc.tile_pool(name="pj", bufs=2, space="PSUM"))
        ps = ctx.enter_context(tc.tile_pool(name="ps", bufs=2, space="PSUM"))
        pacc = ctx.enter_context(tc.tile_pool(name="pacc", bufs=2, space="PSUM"))
        pexp_pool = ctx.enter_context(tc.tile_pool(name="pexp", bufs=3))

        # persistent staging for k/v pieces (written once per block, so no
        # ring WAR against the stage DMAs)
        kst_all = singles.tile([H, NMYB, 128], BF16)
        vst_all = singles.tile([128, NMYB, 65], BF16)
        nc.vector.memset(vst_all[:, :, H : H + 1], 1.0)

        def proj_piece(p):
            """Project q/k/v for my token blocks of piece p and stage k/v."""
            b0, nb = PIECES[p]
            lo = 128 * b0
            for half in range(0, nb, 2):
                lo2 = lo + 128 * half
                bb = b0 + half
                # one PSUM tile per 256 tokens: [qk 0:256 | v_b0 | v_b1]
                pjt = pj.tile([128, 384], F32, tag="pj", name="pjt")
                for ch in range(NCH):
                    nc.tensor.matmul(
                        pjt[:, 0:256], lhsT=wall_sb[:, ch, 0 : 2 * H],
                        rhs=xt_all[:, ch, lo2 : lo2 + 256],
                        start=(ch == 0), stop=(ch == NCH - 1),
                    )
                for b2 in range(2):
                    for ch in range(NCH):
                        nc.tensor.matmul(
                            pjt[:, 256 + 64 * b2 : 320 + 64 * b2],
                            lhsT=xt_all[:, ch, lo2 + 128 * b2 : lo2 + 128 * b2 + 128],
                            rhs=wall_sb[:, ch, 2 * H : 3 * H],
                            start=(ch == 0), stop=(ch == NCH - 1),
                        )
                nc.vector.tensor_scalar_add(
                    qT_sb[:, bb : bb + 2, :], pjt[0:H, 0:256],
                    bqk_sb[0:H, :],
                )
                nc.vector.tensor_scalar_add(
                    kst_all[:, bb : bb + 2, :], pjt[H : 2 * H, 0:256],
                    bqk_sb[H : 2 * H, :],
                )
                nc.vector.tensor_add(
                    vst_all[:, bb, 0:H], pjt[:, 256:320], bv_bc
                )
                nc.vector.tensor_add(
                    vst_all[:, bb + 1, 0:H], pjt[:, 320:384], bv_bc
                )

            # stage to DRAM (sync queue; kept off the scalar queue so exp
            # never sits behind a waiting DMA)
            k_dst = kv_in[p][:, 0 : _nk(nb)].rearrange("o (q s) -> (o q) s", q=H)
            v_dst = kv_in[p][:, _nk(nb) : _nx(nb)].rearrange(
                "o (q s) -> (o q) s", q=128
            )
            nc.sync.dma_start(out=k_dst, in_=kst_all[:, b0 : b0 + nb, :])
            nc.sync.dma_start(out=v_dst, in_=vst_all[:, b0 : b0 + nb, :])

        def exchange_cc(p):
            """Pair-AllGather of k/v piece p (gpsimd queue)."""
            if with_cc:
                nc.gpsimd.collective_compute(
                    "AllGather",
                    mybir.AluOpType.bypass,
                    replica_groups=[[0, 1], [2, 3], [4, 5], [6, 7]],
                    ins=[kv_in[p][:, :]],
                    outs=[kv_red[p][:, :]],
                )
            else:
                # model the gather as a full-size local copy (258*2B runs)
                nb = PIECES[p][1]
                src = bass.AP(
                    tensor=kv_in[p][:, :].tensor,
                    offset=kv_in[p][:, :].offset,
                    ap=[[0, 2], [258, 64 * nb], [1, 258]],
                )
                nc.gpsimd.dma_start(
                    out=kv_red[p][:, :].rearrange("r (q s) -> r q s", s=258),
                    in_=src,
                )

        def readback(p):
            """Read AllGather piece p back into the rank-indexed k/v slots."""
            b0, nb = PIECES[p]
            k_src = kv_red[p][:, 0 : _nk(nb)].rearrange("r (q s) -> q r s", q=H)
            v_src = kv_red[p][:, _nk(nb) : _nx(nb)].rearrange(
                "r (q s) -> q r s", q=128
            )
            nc.sync.dma_start(
                out=kT_sb[:, :, b0 : b0 + nb, :], in_=k_src
            )
            nc.sync.dma_start(
                out=v_sb[:, :, b0 : b0 + nb, :], in_=v_src
            )

        # ---- attention pipeline ----
        # flat group list; groups gather into [128, GPT*128] PSUM tiles
        cur = {"ps": None, "n": 0, "meta": []}
        deferred = []  # (ps_tile, pexp_tile, meta) awaiting out-matmul emission
        acc_of = {}  # q block -> psum accumulator tile (pair-granular ring)

        def get_acc(i):
            if i not in acc_of:
                acc_of[i] = pacc.tile([128, 2, 66], F32, tag="pacc", name="acc_t")
                acc_of[i + 1] = acc_of[i]
            return acc_of[i]

        def emit_out(batch):
            """Out matmuls (and norms) for a completed exp tile."""
            ps_t, px_t, meta = batch
            for g, (i, r, j) in enumerate(meta):
                nc.tensor.matmul(
                    acc_of[i][:, i % 2, 0:65],
                    lhsT=px_t[:, g, :],
                    rhs=v_sb[:, r, j, :],
                    start=(r == 0 and j == 0),
                    stop=(r == 1 and j == i),
                )
                if r == 1 and j == i:
                    nc.vector.reciprocal(
                        rcp_sb[:, i : i + 1], acc_of[i][:, i % 2, 64:65]
                    )
                    nc.vector.tensor_scalar_mul(
                        out_stage[:, i, :], acc_of[i][:, i % 2, 0:64],
                        rcp_sb[:, i : i + 1],
                    )
                    if i % 4 == 3:
                        g4 = i // 4
                        nc.gpsimd.dma_start(
                            out=out3[:, 4 * g4 : 4 * g4 + 4, :],
                            in_=out_stage[:, 4 * g4 : 4 * g4 + 4, :],
                        )

        def flush_tile():
            """Close the current scores tile: exp it, queue its out matmuls."""
            if cur["n"] == 0:
                return
            ps_t, meta = cur["ps"], cur["meta"]
            px_t = pexp_pool.tile([128, GPT, 128], BF16, tag="px", name="px_t")
            n = cur["n"]
            nc.scalar.activation(
                px_t[:, 0:n, :], ps_t[:, 0:n, :],
                mybir.ActivationFunctionType.Exp, scale=SCALE,
            )
            deferred.append((ps_t, px_t, list(meta)))
            cur["ps"], cur["n"], cur["meta"] = None, 0, []
            if len(deferred) > 1:
                emit_out(deferred.pop(0))

        def group(i, r, j):
            """Scores (+mask) for q block i vs k slot (r, j)."""
            if cur["ps"] is None:
                cur["ps"] = ps.tile([128, GPT, 128], F32, tag="ps", name="ps_t")
            g = cur["n"]
            ps_t = cur["ps"]
            masked = j == i
            nc.tensor.matmul(
                ps_t[:, g, :], lhsT=kT_sb[:, r, j, :], rhs=qT_sb[:, i, :],
                start=True, stop=not masked,
            )
            if masked:
                nc.tensor.matmul(
                    ps_t[:, g, :], lhsT=ident_sb, rhs=masks_sb[:, r, :],
                    start=False, stop=True,
                )
            cur["meta"].append((i, r, j))
            cur["n"] += 1
            if cur["n"] == GPT:
                flush_tile()

        def wave(w):
            """Attention for my q blocks 2w, 2w+1 (k slots j <= i ready)."""
            get_acc(2 * w)
            for i in (2 * w, 2 * w + 1):
                for j in range(i + 1):
                    for r in range(2):
                        group(i, r, j)

        # ---- pipelined emission ----
        # readback(p) trails exchange_cc(p) by one piece so its wait on the
        # collective is satisfied before it reaches the DMA queue head;
        # wave(w) trails readback(w) the same way.
        x_dma(0)
        x_dma(1)

        # PE warmup: dependency-free matmuls so the tensor engine is at full
        # p-state when proj 0's data lands
        wps = ps.tile([128, GPT, 128], F32, tag="ps", name="wps")
        for wi in range(16):
            nc.tensor.matmul(
                wps[:, wi % GPT, :], lhsT=ident_sb, rhs=ident_sb,
                start=True, stop=True,
            )

        # Interleave: the framework tracks k/v SBUF deps at tile granularity,
        # so wave(w) must be emitted after readback(w) but BEFORE readback
        # (w+1), or its scores inherit waits on later pieces.  x DMAs share
        # the sync queue with the stage/readback hops so the FIFO DMA bus
        # serves the latency-critical hops in between x pieces.
        proj_piece(0)
        exchange_cc(0)
        x_dma(2)
        proj_piece(1)
        exchange_cc(1)
        readback(0)
        x_dma(3)
        proj_piece(2)
        exchange_cc(2)
        wave(0)
        flush_tile()  # don't hold wave 0's 6 groups hostage to wave 1 data
        readback(1)
        x_dma(4)
        proj_piece(3)
        exchange_cc(3)
        wave(1)
        readback(2)
        proj_piece(4)
        exchange_cc(4)
        wave(2)
        readback(3)
        wave(3)
        readback(4)
        wave(4)
        wave(5)
        wave(6)
        wave(7)
        flush_tile()
        while deferred:
            emit_out(deferred.pop(0))

    nc.finalize()
    return nc


_PROGRAM_CACHE = {}


def _get_program():
    if "prog" not in _PROGRAM_CACHE:
        _PROGRAM_CACHE["prog"] = build_program()
    return _PROGRAM_CACHE["prog"]


def _bf16(a):
    import ml_dtypes
    return np.asarray(a).astype(ml_dtypes.bfloat16)


def _make_masks(parity: int) -> np.ndarray:
    """[2, 128, 128] additive mask tiles for the j == i k slot, per rank.

    q block i is natural block n = 2i + parity; rank-r slot i is natural
    block 2i + r.  r == parity → the diagonal block (strict lower triangle
    of scores^T masked: k row p > q col j).  Otherwise r=0 is fully valid
    (natural 2i < n, parity 1) and r=1 fully masked (2i+1 > n, parity 0).
    """
    p = np.arange(128)[:, None]
    j = np.arange(128)[None, :]
    tri = np.where(p > j, MASK_VAL, 0.0).astype(np.float32)
    full = np.full((128, 128), MASK_VAL, np.float32)
    zero = np.zeros((128, 128), np.float32)
    if parity == 0:
        m = np.stack([tri, full])
    else:
        m = np.stack([zero, tri])
    return _bf16(m)


def kernel(x, Wq, bq, Wk, bk, Wv, bv):
    x = np.asarray(x, dtype=np.float32)
    wall = np.zeros((D, 256), np.float32)
    wall[:, 0:H] = np.asarray(Wq)
    wall[:, H : 2 * H] = np.asarray(Wk)
    wall[:, 2 * H : 3 * H] = np.asarray(Wv)
    wall = _bf16(wall)
    bqk = np.concatenate(
        [np.asarray(bq), np.asarray(bk)]
    ).astype(np.float32).reshape(2 * H, 1)
    bv_ = np.asarray(bv, dtype=np.float32).reshape(1, H)
    ident = _bf16(np.eye(128, dtype=np.float32))

    nc = _get_program()

    in_maps = []
    for core in range(NCORES):
        b, parity = core // 2, core % 2
        blocks = list(range(parity, 32, 2))
        xTp = np.ascontiguousarray(
            x[b].T.reshape(D, 32, 128)[:, blocks, :].reshape(D, S // 2)
        )
        in_maps.append(
            {
                "xT": _bf16(xTp),
                "wall": wall,
                "bqk": bqk,
                "bv": bv_,
                "masks": _make_masks(parity),
                "ident": ident,
            }
        )

    res = run_bass_kernel_spmd(nc, in_maps, list(range(NCORES)))

    out = np.empty((B, S, H), np.float32)
    for core in range(NCORES):
        b, parity = core // 2, core % 2
        o = np.asarray(res.results[core]["out"], np.float32).reshape(NMYB, 128, H)
        out[b].reshape(32, 128, H)[parity::2] = o
    return out
